# revision 21
# baseline (speedup 1.0000x reference)
"""Trainium2 Bass kernel for 4-head spatial attention + BatchNorm (dense_transformer).

Reference computation (per batch b, with n = 64*64 = 4096 spatial positions):
  qkv = W_qkv @ x            (1x1 conv == channel matmul)
  scores = (q*scale)^T k per head (head_dim 32), softmax over keys
  out = attn @ v^T ; y = W_out @ out + b_out ; BatchNorm2d over (batch, spatial)

Sharding: 8 cores = (batch b = core//2) x (n-half = core%2). Each core computes
its 2048 query positions against all 4096 keys for all 4 heads, producing the
full 256-channel output for its (b, n-half) shard. BatchNorm statistics are
all-reduced (2KB) across all 8 cores. b_out cancels inside BatchNorm and is
skipped.

Layout core ideas:
 - scores computed TRANSPOSED: scT[m, n] = sum_d k[d,m] q[d,n] so softmax's
   reduction axis (m=keys) is the PE contraction axis downstream, never needing
   an on-chip transpose of the 4096x2048 attention matrix.
 - exp is unnormalized (no max subtraction; |scaled scores| <= ~6) and the
   denominators come from a ones-vector matmul; division happens on the tiny
   [128, 512] av accumulator instead of the big attention matrix.
 - K=32 score matmuls are packed 2-at-a-time into PE row-groups
   (tile_position), av/den matmuls 4-heads-at-a-time into PE col-groups.
"""

import numpy as np
import ml_dtypes

import concourse.bass as bass
import concourse.tile as tile
from concourse import bacc, mybir
from concourse.bass_utils import run_bass_kernel_spmd

BF16 = mybir.dt.bfloat16
F32 = mybir.dt.float32
I16 = mybir.dt.int16
AF = mybir.ActivationFunctionType
ALU = mybir.AluOpType

B, C, HW, N = 4, 256, 64, 4096
HEADS, DH, HID = 4, 32, 128
NLOC = N // 2          # 2048 query positions per core
NJ = NLOC // 512       # 4 n-chunks of 512
MT = N // 128          # 32 key tiles of 128
SCALE = float(DH) ** -0.5
EPS = 1e-5
NCORES = 8

# Schraudolph-style exp for the VectorE path: bf16's bit pattern for 2^x is
# ~ 128*(x + 127 - c) in int16 space, so exp(s*SCALE) ~= bitcast_bf16(
# int16(round(EXP_ALPHA*s + EXP_BETA))). One DVE tensor_scalar (mult+add with
# int16 output) replaces the ScalarE exp for ~45% of the attention tiles,
# halving the ScalarE bottleneck. Max per-element rel err ~3%; softmax
# normalization cancels most of it (end-to-end contribution ~2e-3).
L2E = 1.4426950408889634
EXP_ALPHA = 128.0 * SCALE * L2E
EXP_C = 0.043750
EXP_BETA = 128.0 * (127.0 - EXP_C)
# fp32 round-to-int via the 1.5*2^23 magic constant: t = s*EXP_ALPHA +
# EXP_MAGIC_B keeps the rounded integer exp2 bit pattern in the LOW 16 bits
# of the f32 result — read back with a stride-2 bf16 bitcast view. Avoids
# the DVE's slow float->int conversion path entirely.
EXP_MAGIC_B = 12599162.0


def build_nc(nj_count=NJ, mt_count=MT, do_coll=True, stage="full", reps=1,
             dve_mod=(16, 15), dve_kind="magic", scb=3, avb=1, look=1,
             scalar_evac=False, act_stats=True, out_bf16=True):
    nc = bacc.Bacc("TRN2", target_bir_lowering=False)

    xq_d = nc.declare_dram_parameter("xq", [C, NLOC], BF16, isOutput=False)
    xkv_d = nc.declare_dram_parameter("xkv", [C, N], BF16, isOutput=False)
    wqT_d = nc.declare_dram_parameter("wqT", [C, HID], BF16, isOutput=False)
    wkT_d = nc.declare_dram_parameter("wkT", [C, HID], BF16, isOutput=False)
    wvT_d = nc.declare_dram_parameter("wvT", [C, HID], BF16, isOutput=False)
    woT_d = nc.declare_dram_parameter("woT", [HID, C], BF16, isOutput=False)
    gb_d = nc.declare_dram_parameter("gb", [128, 4], F32, isOutput=False)
    out_dt = BF16 if out_bf16 else F32
    out_d = nc.declare_dram_parameter("out", [C, NLOC], out_dt, isOutput=True)

    with tile.TileContext(nc) as tc:
        with (
            tc.tile_pool(name="consts", bufs=1) as consts,
            tc.tile_pool(name="acts", bufs=1) as acts,
            tc.tile_pool(name="expp", bufs=4) as expp,
            tc.tile_pool(name="norm", bufs=2) as normp,
            tc.tile_pool(name="dram", bufs=1, space="DRAM") as dram,
        ):
            # ---- persistent SBUF tensors ----
            wq_sb = consts.tile([128, 2 * HID], BF16)   # [c-chunk(2) x 128]
            wk_sb = consts.tile([128, 2 * HID], BF16)
            wv_sb = consts.tile([128, 2 * HID], BF16)
            wo_sb = consts.tile([128, C], BF16)
            gb_sb = consts.tile([128, 4], F32)
            ones_sb = consts.tile([128, 32], BF16)
            eps_sb = consts.tile([128, 1], F32)
            zrow_sb = consts.tile([128, 512], BF16)

            xq_sb = acts.tile([128, 2 * NLOC], BF16)    # col = cc*NLOC + n
            xkv_sb = acts.tile([128, 2 * N], BF16)      # col = cc*N + m
            q_sb = acts.tile([128, NLOC], BF16)         # part = h*32+d
            k_sb = acts.tile([128, N], BF16)            # part = h*32+d
            vT_sb = acts.tile([128, N], BF16)           # col = mt*128 + h*32 + d
            avn_sb = acts.tile([128, NLOC], BF16)       # normalized attn@v, part=h*32+d
            y_sb = acts.tile([128, 2 * NLOC], F32)      # col = ct*NLOC + n
            sq_sb = acts.tile([128, NLOC], F32)         # scratch for squared y
            stats_sb = acts.tile([128, 4], F32)
            statsr_sb = acts.tile([128, 4], F32)

            stats_in = dram.tile([128, 4], F32)
            stats_out = dram.tile([128, 4], F32)

            # ---- load inputs ----
            for cc in range(2):
                nc.sync.dma_start(
                    out=xq_sb[:, cc * NLOC:(cc + 1) * NLOC],
                    in_=xq_d[cc * 128:(cc + 1) * 128, :],
                )
                nc.sync.dma_start(
                    out=xkv_sb[:, cc * N:(cc + 1) * N],
                    in_=xkv_d[cc * 128:(cc + 1) * 128, :],
                )
                nc.sync.dma_start(
                    out=wq_sb[:, cc * HID:(cc + 1) * HID],
                    in_=wqT_d[cc * 128:(cc + 1) * 128, :],
                )
                nc.sync.dma_start(
                    out=wk_sb[:, cc * HID:(cc + 1) * HID],
                    in_=wkT_d[cc * 128:(cc + 1) * 128, :],
                )
                nc.sync.dma_start(
                    out=wv_sb[:, cc * HID:(cc + 1) * HID],
                    in_=wvT_d[cc * 128:(cc + 1) * 128, :],
                )
            nc.sync.dma_start(out=wo_sb[:], in_=woT_d[:])
            nc.sync.dma_start(out=gb_sb[:], in_=gb_d[:])
            nc.vector.memset(ones_sb[:], 1.0)
            nc.vector.memset(eps_sb[:], EPS)
            nc.vector.memset(zrow_sb[:], 0.0)

            # ---- q/k projections: out part = head-stacked channel ----
            # Evacuations ride ScalarE (idle in this phase; DVE handles the
            # in-loop magic-exp tiles) unless scalar_evac is off.
            evac = nc.scalar.copy if scalar_evac else nc.vector.tensor_copy
            with tc.tile_pool(name="mmps", bufs=2, space="PSUM") as mmps:
                # ~5us of K=1 dummy matmuls during the input-DMA wait: warms
                # the PE HAM clock gate (4/8 -> 8/8) deterministically on
                # every core before the projections, which also aligns the
                # cores' timelines for the BN-stats AllReduce at the end.
                warm_ps = mmps.tile([128, 512], F32, tag="warm")
                for _ in range(12):
                    nc.tensor.matmul(
                        warm_ps[:], lhsT=zrow_sb[0:1, 0:128],
                        rhs=zrow_sb[0:1, :],
                        start=True, stop=True, skip_group_check=True,
                    )
                for j in range(NJ):
                    q_ps = mmps.tile([128, 512], F32, tag="qk")
                    for cc in range(2):
                        nc.tensor.matmul(
                            q_ps[:],
                            lhsT=wq_sb[:, cc * HID:(cc + 1) * HID],
                            rhs=xq_sb[:, cc * NLOC + j * 512: cc * NLOC + j * 512 + 512],
                            start=(cc == 0), stop=(cc == 1),
                        )
                    evac(q_sb[:, j * 512:(j + 1) * 512], q_ps[:])
                for j in range(N // 512):
                    k_ps = mmps.tile([128, 512], F32, tag="qk")
                    for cc in range(2):
                        nc.tensor.matmul(
                            k_ps[:],
                            lhsT=wk_sb[:, cc * HID:(cc + 1) * HID],
                            rhs=xkv_sb[:, cc * N + j * 512: cc * N + j * 512 + 512],
                            start=(cc == 0), stop=(cc == 1),
                        )
                    evac(k_sb[:, j * 512:(j + 1) * 512], k_ps[:])
                # vT: x as stationary -> out [m-tile, v-channel]; 4 m-tiles
                # share one PSUM bank so the evacuation runs 512 wide.
                for mtg in range(MT // 4):
                    v_ps = mmps.tile([128, 512], F32, tag="vt")
                    for sub in range(4):
                        mt = mtg * 4 + sub
                        for cc in range(2):
                            nc.tensor.matmul(
                                v_ps[:, sub * 128:(sub + 1) * 128],
                                lhsT=xkv_sb[:, cc * N + mt * 128: cc * N + mt * 128 + 128],
                                rhs=wv_sb[:, cc * HID:(cc + 1) * HID],
                                start=(cc == 0), stop=(cc == 1),
                            )
                    nc.vector.tensor_copy(vT_sb[:, mtg * 512:(mtg + 1) * 512], v_ps[:])

            so = {"qkv": 0, "attn": 1, "yproj": 2, "full": 3}[stage]
            if so == 0:
                nc.gpsimd.dma_start(out=out_d[0:128, :], in_=q_sb[:])
            # ---- attention ----
            if nj_count < NJ and so >= 1:  # debug truncation: keep reads defined
                nc.vector.memset(avn_sb[:], 0.0)
                nc.vector.memset(y_sb[:], 0.0)
            with (
                tc.tile_pool(name="scps", bufs=scb, space="PSUM") as scps,
                tc.tile_pool(name="avps", bufs=avb, space="PSUM") as avps,
            ):
                # Phase-batched per (j, mt): all 4 heads' score matmuls issue
                # together (4 concurrent PE row-strips), then ScalarE exps
                # duet 0 while VectorE magic-exps duet 1 IN PARALLEL, then the
                # pending av phase (4 concurrent col-strips) and den phase
                # (4 col-strips). Same-type matmuls never conflict on the
                # 32x32 subarray grid, so each phase runs ~one MM duration —
                # the PE stays ~fully busy (HAM warm) and both exp engines
                # overlap instead of alternating.
                def magic_rhs(ex_, hh):
                    # low 16 bits of each f32 = the exp2 bf16 pattern
                    return ex_[:].bitcast(BF16).rearrange(
                        "p (h n two) -> p h n two", h=2, two=2
                    )[:, hh:hh + 1, :, 0:1]

                def emit_avden(st):
                    ex0_, ex1_, dve_, av_, den_, mt_, last_, norm_, j_ = st
                    def rhs_for(h):
                        hh = h % 2
                        if h >= 2 and dve_:
                            return magic_rhs(ex1_, hh)
                        ex_ = ex0_ if h < 2 else ex1_
                        return ex_[:, hh * 512:(hh + 1) * 512]
                    for h in range(4):
                        nc.tensor.matmul(
                            av_[32 * h:32 * h + 32, :],
                            lhsT=vT_sb[:, mt_ * 128 + 32 * h: mt_ * 128 + 32 * h + 32],
                            rhs=rhs_for(h),
                            start=False, stop=last_,
                            tile_position=(0, 32 * h),
                            skip_group_check=True,
                        )
                    # M=32 ones-matmuls: the softmax denominator, broadcast
                    # across each head's 32 partitions
                    for h in range(4):
                        nc.tensor.matmul(
                            den_[32 * h:32 * h + 32, :],
                            lhsT=ones_sb[:, 0:32],
                            rhs=rhs_for(h),
                            start=False, stop=last_,
                            tile_position=(0, 32 * h),
                            skip_group_check=True,
                        )
                    if norm_:
                        # normalize as soon as the accumulators close, before
                        # their pool slots can be recycled: avn = av * (1/den)
                        rden_sb = normp.tile([128, 512], F32, tag="rden_sb")
                        nc.vector.reciprocal_approx_fast(
                            out=rden_sb[:], in_=den_[:]
                        )
                        nc.vector.tensor_mul(
                            avn_sb[:, j_ * 512:(j_ + 1) * 512], av_[:], rden_sb[:]
                        )

                pending = []  # avden of mt i-1 issues after mt i's exps
                for rep_j in range((nj_count * reps) if so >= 1 else 0):
                    j = rep_j % nj_count
                    av_ps = avps.tile([128, 512], F32, tag="av")
                    den_ps = avps.tile([128, 512], F32, tag="den")
                    # open one whole-bank accumulation group per accumulator
                    # with a K=1 zero-weights matmul; the col-tiled matmuls
                    # accumulate into it (start=False), so no overlapping
                    # has_written clears can race.
                    nc.tensor.matmul(
                        av_ps[:], lhsT=zrow_sb[0:1, 0:128], rhs=zrow_sb[0:1, :],
                        start=True, stop=False, skip_group_check=True,
                    )
                    nc.tensor.matmul(
                        den_ps[:], lhsT=zrow_sb[0:1, 0:128], rhs=zrow_sb[0:1, :],
                        start=True, stop=False, skip_group_check=True,
                    )
                    for mt in range(mt_count):
                        sc0 = scps.tile([128, 1024], F32, tag="sc")
                        sc1 = scps.tile([128, 1024], F32, tag="sc")
                        for dp in range(2):  # head duets {0,1}, {2,3}
                            sc = sc0 if dp == 0 else sc1
                            for hh in range(2):
                                h = dp * 2 + hh
                                nc.tensor.matmul(
                                    sc[:, hh * 512:(hh + 1) * 512],
                                    lhsT=k_sb[32 * h:32 * h + 32, mt * 128:(mt + 1) * 128],
                                    rhs=q_sb[32 * h:32 * h + 32, j * 512:(j + 1) * 512],
                                    start=True, stop=True,
                                    tile_position=(32 * h, 0),
                                )
                        ex0 = expp.tile([128, 1024], BF16, tag="ex")
                        nc.scalar.activation(ex0[:], sc0[:], AF.Exp, scale=SCALE)
                        use_dve = (mt % dve_mod[0]) < dve_mod[1]
                        if use_dve:
                            ex1 = expp.tile([128, 1024], F32, tag="exm")
                            nc.vector.tensor_scalar(
                                out=ex1[:], in0=sc1[:],
                                scalar1=EXP_ALPHA, scalar2=EXP_MAGIC_B,
                                op0=ALU.mult, op1=ALU.add,
                            )
                        else:
                            ex1 = expp.tile([128, 1024], BF16, tag="ex")
                            nc.scalar.activation(ex1[:], sc1[:], AF.Exp, scale=SCALE)
                        if len(pending) >= look:
                            emit_avden(pending.pop(0))
                        pending.append((
                            ex0, ex1, use_dve, av_ps, den_ps, mt,
                            mt == mt_count - 1, mt == mt_count - 1, j,
                        ))
                    # flush before the next j's accumulator-opening matmuls
                    # reuse the av/den banks (required when avb == 1)
                    while pending:
                        emit_avden(pending.pop(0))

            if so == 1:
                nc.gpsimd.dma_start(out=out_d[0:128, :], in_=avn_sb[:])
            # ---- output projection: y[ct*128 + o, n] ----
            # With act_stats, the PSUM->SBUF evacuation runs on ScalarE with
            # accum_out producing per-chunk column sums of y (and, via a second
            # Square pass into a scratch tile, of y^2) — the BatchNorm sums
            # come out of the copies for free instead of costing DVE reduces.
            ysum_sb = acts.tile([128, 8], F32)
            ysq_sb = acts.tile([128, 8], F32)
            ysc_sb = acts.tile([128, 512], BF16)
            with tc.tile_pool(name="yps", bufs=2, space="PSUM") as yps:
                for j in range(NJ if so >= 2 else 0):
                    for ct in range(2):
                        y_ps = yps.tile([128, 512], F32, tag="y")
                        nc.tensor.matmul(
                            y_ps[:],
                            lhsT=wo_sb[:, ct * 128:(ct + 1) * 128],
                            rhs=avn_sb[:, j * 512:(j + 1) * 512],
                            start=True, stop=True,
                        )
                        ysl = y_sb[:, ct * NLOC + j * 512: ct * NLOC + j * 512 + 512]
                        sidx = ct * 4 + j
                        if act_stats:
                            # ScalarE evacuates + sums y; DVE squares+reduces
                            # the evacuated chunk in parallel
                            nc.scalar.activation(
                                ysl, y_ps[:], AF.Copy,
                                accum_out=ysum_sb[:, sidx:sidx + 1],
                            )
                            nc.vector.tensor_mul(sq_sb[:, 0:512], ysl, ysl)
                            nc.vector.tensor_reduce(
                                ysq_sb[:, sidx:sidx + 1], sq_sb[:, 0:512],
                                axis=mybir.AxisListType.X, op=ALU.add,
                            )
                        else:
                            nc.vector.tensor_copy(ysl, y_ps[:])

            if so == 2:
                nc.sync.dma_start(out=out_d[0:128, :], in_=y_sb[:, 0:NLOC])
            # ---- BatchNorm stats (b_out cancels in BN) ----
            if act_stats:
                for ct in range(2 if so >= 3 else 0):
                    nc.vector.tensor_reduce(
                        stats_sb[:, ct:ct + 1], ysum_sb[:, ct * 4:ct * 4 + 4],
                        axis=mybir.AxisListType.X, op=ALU.add,
                    )
                    nc.vector.tensor_reduce(
                        stats_sb[:, 2 + ct:3 + ct], ysq_sb[:, ct * 4:ct * 4 + 4],
                        axis=mybir.AxisListType.X, op=ALU.add,
                    )
            else:
                for ct in range(2 if so >= 3 else 0):
                    ysl = y_sb[:, ct * NLOC:(ct + 1) * NLOC]
                    nc.vector.tensor_reduce(
                        stats_sb[:, ct:ct + 1], ysl, axis=mybir.AxisListType.X, op=ALU.add
                    )
                    nc.vector.tensor_mul(sq_sb[:], ysl, ysl)
                    nc.vector.tensor_reduce(
                        stats_sb[:, 2 + ct:3 + ct], sq_sb[:],
                        axis=mybir.AxisListType.X, op=ALU.add,
                    )
            inv_n = 1.0 / float(B * N)
            statsn_sb = acts.tile([128, 4], F32)
            if so >= 3:
                # pre-scale by 1/n so the AllReduce returns mean/E[y^2]
                # directly — shortens the post-collective critical path
                nc.vector.tensor_scalar_mul(statsn_sb[:], stats_sb[:], inv_n)
            if do_coll and so >= 3:
                nc.gpsimd.dma_start(out=stats_in[:], in_=statsn_sb[:])
                nc.gpsimd.collective_compute(
                    "AllReduce",
                    ALU.add,
                    replica_groups=[list(range(NCORES))],
                    ins=[stats_in.opt()],
                    outs=[stats_out.opt()],
                )
                nc.gpsimd.dma_start(out=statsr_sb[:], in_=stats_out[:])
            elif so >= 3:
                nc.vector.tensor_scalar_mul(statsr_sb[:], statsn_sb[:], 8.0)

            tmp_sb = consts.tile([128, 2], F32)
            var_sb = consts.tile([128, 2], F32)
            std_sb = consts.tile([128, 2], F32)
            rstd_sb = consts.tile([128, 2], F32)
            scal_sb = consts.tile([128, 2], F32)
            bias_sb = consts.tile([128, 2], F32)
            if so >= 3:
                mean_ap = statsr_sb[:, 0:2]
                nc.vector.tensor_mul(tmp_sb[:], mean_ap, mean_ap)
                nc.vector.tensor_sub(var_sb[:], statsr_sb[:, 2:4], tmp_sb[:])
                nc.scalar.activation(
                    std_sb[:], var_sb[:], AF.Sqrt, bias=eps_sb[:, 0:1]
                )
                nc.vector.reciprocal(rstd_sb[:], std_sb[:])
                nc.vector.tensor_mul(scal_sb[:], gb_sb[:, 0:2], rstd_sb[:])
                nc.vector.tensor_mul(tmp_sb[:], mean_ap, scal_sb[:])
                nc.vector.tensor_sub(bias_sb[:], gb_sb[:, 2:4], tmp_sb[:])

            # ---- apply: out = y*scale + bias, then store ----
            for ct in range(2 if so >= 3 else 0):
                yo = normp.tile([128, NLOC], out_dt, tag="yo")
                nc.vector.tensor_scalar(
                    out=yo[:],
                    in0=y_sb[:, ct * NLOC:(ct + 1) * NLOC],
                    scalar1=scal_sb[:, ct:ct + 1],
                    scalar2=bias_sb[:, ct:ct + 1],
                    op0=ALU.mult, op1=ALU.add,
                )
                nc.sync.dma_start(
                    out=out_d[ct * 128:(ct + 1) * 128, :], in_=yo[:]
                )

    nc.compile()
    return nc


_NC_CACHE = {}


def _get_nc():
    if "nc" not in _NC_CACHE:
        _NC_CACHE["nc"] = build_nc()
    return _NC_CACHE["nc"]


def kernel(x, W_qkv, W_out, b_out, gamma, beta):
    bf16 = ml_dtypes.bfloat16
    x = np.asarray(x, np.float32)
    W_qkv = np.asarray(W_qkv, np.float32)
    W_out = np.asarray(W_out, np.float32)
    gamma = np.asarray(gamma, np.float32)
    beta = np.asarray(beta, np.float32)

    xf = x.reshape(B, C, N)
    wqT = np.ascontiguousarray(W_qkv[0:HID, :].T).astype(bf16)
    wkT = np.ascontiguousarray(W_qkv[HID:2 * HID, :].T).astype(bf16)
    wvT = np.ascontiguousarray(W_qkv[2 * HID:3 * HID, :].T).astype(bf16)
    woT = np.ascontiguousarray(W_out.T).astype(bf16)
    gb = np.stack(
        [gamma[0:128], gamma[128:256], beta[0:128], beta[128:256]], axis=1
    ).astype(np.float32)
    gb = np.ascontiguousarray(gb)

    in_maps = []
    for core in range(NCORES):
        b, half = core // 2, core % 2
        xb = np.ascontiguousarray(xf[b]).astype(bf16)
        xq = np.ascontiguousarray(xb[:, half * NLOC:(half + 1) * NLOC])
        in_maps.append({
            "xq": xq, "xkv": xb,
            "wqT": wqT, "wkT": wkT, "wvT": wvT, "woT": woT, "gb": gb,
        })

    nc = _get_nc()
    _NC_CACHE["last_in_maps"] = in_maps
    res = run_bass_kernel_spmd(nc, in_maps, core_ids=list(range(NCORES)))

    out = np.empty((B, C, N), np.float32)
    for core in range(NCORES):
        b, half = core // 2, core % 2
        out[b][:, half * NLOC:(half + 1) * NLOC] = np.asarray(
            res.results[core]["out"]
        ).astype(np.float32)
    return out.reshape(B, C, HW, HW)

